# revision 2
# baseline (speedup 1.0000x reference)
"""Trainium2 Bass kernel for nn_CIN (xDeepFM compressed-interaction network).

Math: each CIN layer computes, per sample b and feature-dim d (a "column"
n=(b,d)):  y[o] = sum_{h,m} W[o,h,m] * a[h] * b[m]  — a bilinear form,
factorized by polarization (a*b = ((a+b)^2 - a^2 - b^2)/2) into
  s = V @ t  (pair sums),  q = s*s,  y = C @ q + G @ t^2
with V a 0/1 pair-selection matrix and C/G folded from W host-side.

V passes run as fp8e4 DoubleRow matmuls (0.5 cycles per output element —
2x fp16): the selection matrix is duplicated in both k-planes and the
moving operand is split into fp8 hi/lo planes (t = t_hi + t_lo, both
e4m3; the PE sums the planes, recovering x to ~0.1%).  nh = relu(y0) is
cast to a single fp8 plane (lo plane stays zero); C/G passes and q stay
fp16 — fp8 q or single-fp8 x costs 3-4e-2 rel err, over the 2e-2 budget
(verified by exact simulation; this config measures 1.71e-2).
Biases ride an all-ones row in the x^2 tile against a b-row in G0T/G1T.

PE per tile: 26 DR passes (107 ns) + 28 fp16 passes (213 ns) = 8.75 us.
DR V passes are too fast to hide the ~1-2 us square latency inside one
iteration, so the pipeline is 2-deep: iteration k runs
  C0(k-1), G0(k-1) -> nh_hi(k-1) | V0(k) | C1(k-2)..G1(k-2) x V1(k-1)
and every C pass consumes squares produced a full iteration earlier.
C chunks are consumed ordered by square engine (ACT first, Pool last) so
PE never queues behind a slow square; the iteration-boundary ring slot
(pair 12) is squared via the fast DVE copy path.

Squares per tile: 13 chunk-pairs [128, 1024] spanning both banks of a
2-bank PSUM ring tile: 8 on ScalarE (activation square), 1 on DVE
(copy + in-place 2x mul), 4 as DVE copy + Pool mul.  relu r0/r1 are DVE
max-ops; d-axis reductions are per-tile DVE tensor_reduce ops.  PSUM:
3-deep ps2 ring (2 banks each) + y0 + y1 = 8 banks exactly.  Engine
budget/tile: PE 8.75, ACT ~8.9, DVE ~8.4, Pool ~8.9 us.

Sharding: pure data parallel — batch 4096 split 512/core across 8 cores;
weights replicated.  Output fp16 -> fp32 on host.
"""

import numpy as np
import ml_dtypes

B, F, D = 4096, 39, 16
L0, L1 = 128, 128
H1 = L0 // 2                      # 64 hidden maps feed layer 1
NCORES = 8
BL = B // NCORES                  # 512 samples per core
NCOL = BL * D                     # 8192 columns per core
NT = 512                          # columns per tile
NTILES = NCOL // NT               # 16
NB = NT // D                      # samples per tile (32)

K0 = F * (F - 1) // 2             # 741 layer-0 pairs
K1 = H1 * F                       # 2496 layer-1 pairs
T1 = 128                          # t rows: [x 0:39 | zeros 39:64 | nh 64:128]
NH0 = 64                          # nh base partition in t
XR = F + 1                        # x^2 rows + ones row (bias)

F8NP = ml_dtypes.float8_e4m3


def _chunks(k):
    out = []
    o = 0
    while o < k:
        c = min(128, k - o)
        out.append((o, c))
        o += c
    return out


CH0 = _chunks(K0)                 # [(0,128)x5, (640,101)]
CH1 = _chunks(K1 + NH0)           # [(0,128)x20] — last 64 rows are nh^2
NC0 = len(CH0)
NC1 = len(CH1)
NP0 = (NC0 + 1) // 2              # 3 square pairs for layer 0
NP1 = (NC1 + 1) // 2              # 10 square pairs for layer 1

# engine per square-pair (global pair index: sq0 pairs 0-2, sq1 pairs 3-12)
# 'A': ScalarE activation-square.  'D': DVE copy + in-place mul.
# 'P': DVE copy + Pool in-place mul (Pool cannot read PSUM).
SQP_ENG = {0: 'A', 1: 'A', 2: 'A', 3: 'P', 4: 'A', 5: 'P', 6: 'A',
           7: 'P', 8: 'A', 9: 'P', 10: 'A', 11: 'A', 12: 'D'}

def _cons_order(nchunks, pair_base):
    # consume chunks squared on ACT first, then DVE, then Pool (slowest)
    order = {'A': 0, 'D': 1, 'P': 2}
    return sorted(range(nchunks),
                  key=lambda i: (order[SQP_ENG[pair_base + i // 2]], i))

CONS0 = None
CONS1 = None

RB = 1                            # tiles per reduce/output block


def _host_weights(W0, b0, W1, b1):
    """Fold W0/W1 into square-trick operands (exact, host-side)."""
    W0 = np.asarray(W0, np.float32)
    W1 = np.asarray(W1, np.float32)
    S0 = W0.reshape(L0, F, F)
    S0 = (S0 + S0.transpose(0, 2, 1)) / 2
    iu = np.triu_indices(F, 1)                       # 741 (h<m) pairs
    V0 = np.zeros((K0, F), np.float32)
    V0[np.arange(K0), iu[0]] = 1
    V0[np.arange(K0), iu[1]] = 1
    C0 = S0[:, iu[0], iu[1]]                         # [128, 741]
    rowsum = S0.sum(2)
    G0 = np.einsum('ohh->oh', S0) * 2 - rowsum       # x^2 coeffs [128, 39]

    B1 = W1.reshape(L1, H1, F)
    hh, mm = np.meshgrid(np.arange(H1), np.arange(F), indexing='ij')
    hh, mm = hh.ravel(), mm.ravel()                  # 2496 pairs, h-major
    V1 = np.zeros((K1 + NH0, T1), np.float32)
    V1[np.arange(K1), mm] = 1                        # x part at rows 0:39
    V1[np.arange(K1), NH0 + hh] = 1                  # nh part at rows 64:128
    V1[K1 + np.arange(NH0), NH0 + np.arange(NH0)] = 1   # nh^2 identity rows
    C1 = np.concatenate([B1[:, hh, mm] / 2,          # [128, 2496]
                         -B1.sum(2) / 2], axis=1)    # nh^2 coeffs [128, 64]
    G1 = (-B1.sum(1) / 2)                            # x^2 coeffs [128, 39]

    def pack_stationary(Ct, chunks):
        out = np.zeros((128, 128 * len(chunks)), np.float16)
        for i, (o, kc) in enumerate(chunks):
            out[:kc, i * 128:i * 128 + 128] = Ct[o:o + kc, :]
        return out

    def pack_v_dr(Vt, chunks, rows):
        # Vt: [rows, K] -> DoubleRow stationary [rows, nchunks*2*128] fp8
        out = np.zeros((rows, 256 * len(chunks)), F8NP)
        for i, (o, kc) in enumerate(chunks):
            blk = Vt[:, o:o + kc].astype(F8NP)
            out[:, i * 256:i * 256 + kc] = blk
            out[:, i * 256 + 128:i * 256 + 128 + kc] = blk
        return out

    def g_with_bias(Gt, b):
        out = np.zeros((XR, 128), np.float16)
        out[:F] = Gt.T.astype(np.float16)
        out[F] = np.asarray(b, np.float32).astype(np.float16)
        return out

    return {
        "V0T": pack_v_dr(V0.T, CH0, F),              # [39, 1536] f8
        "V1T": pack_v_dr(V1.T, CH1, T1),             # [128, 5120] f8
        "C0T": pack_stationary(C0.T, CH0),           # [128, 768] f16
        "C1T": pack_stationary(C1.T, CH1),           # [128, 2560] f16
        "G0T": g_with_bias(G0, b0),                  # [40, 128] f16
        "G1T": g_with_bias(G1, b1),                  # [40, 128] f16
    }


_NC_CACHE = {}


def _build_nc():
    key = "nc"
    if key in _NC_CACHE:
        return _NC_CACHE[key]
    from contextlib import ExitStack
    import concourse.bacc as bacc
    import concourse.mybir as mybir
    import concourse.tile as tile

    f8 = mybir.dt.float8e4
    f16 = mybir.dt.float16
    f32 = mybir.dt.float32
    DR = mybir.MatmulPerfMode.DoubleRow

    nc = bacc.Bacc("TRN2", target_bir_lowering=False, debug=False)

    xhi_d = nc.dram_tensor("xhi", [F, NCOL], f8, kind="ExternalInput")
    xlo_d = nc.dram_tensor("xlo", [F, NCOL], f8, kind="ExternalInput")
    x2o_d = nc.dram_tensor("x2o", [XR, NCOL], f16, kind="ExternalInput")
    V0T_d = nc.dram_tensor("V0T", [F, 256 * NC0], f8, kind="ExternalInput")
    V1T_d = nc.dram_tensor("V1T", [T1, 256 * NC1], f8, kind="ExternalInput")
    C0T_d = nc.dram_tensor("C0T", [128, 128 * NC0], f16, kind="ExternalInput")
    C1T_d = nc.dram_tensor("C1T", [128, 128 * NC1], f16, kind="ExternalInput")
    G0T_d = nc.dram_tensor("G0T", [XR, 128], f16, kind="ExternalInput")
    G1T_d = nc.dram_tensor("G1T", [XR, 128], f16, kind="ExternalInput")
    out_d = nc.dram_tensor("out", [L0 - H1 + L1, BL], f16, kind="ExternalOutput")

    Relu = mybir.ActivationFunctionType.Relu
    Alu = mybir.AluOpType

    with tile.TileContext(nc) as tc, ExitStack() as ctx:
        const = ctx.enter_context(tc.tile_pool(name="const", bufs=1))
        sqp = ctx.enter_context(tc.tile_pool(name="sqp", bufs=2))
        rbp = ctx.enter_context(tc.tile_pool(name="rbp", bufs=2))
        redp = ctx.enter_context(tc.tile_pool(name="redp", bufs=2))
        sps = ctx.enter_context(tc.tile_pool(name="sps", bufs=3, space="PSUM"))
        yps = ctx.enter_context(tc.tile_pool(name="yps", bufs=1, space="PSUM"))

        def emit_square_pair(sq, pcols, ps2, gpair):
            eng = SQP_ENG[gpair]
            dst = sq[0:128, pcols]
            if eng == 'A':
                nc.scalar.square(dst, ps2[0:128, :])
            elif eng == 'D':
                nc.vector.tensor_copy(dst, ps2[0:128, :])
                nc.vector.tensor_mul(dst, dst, dst)
            else:
                nc.vector.tensor_copy(dst, ps2[0:128, :])
                nc.gpsimd.tensor_mul(dst, dst, dst)

        def emit_reduce(out_ap, rblk):
            # one DVE reduce over a whole RB-tile block
            with nc.allow_low_precision(reason="16-term d-sum fits fp16"):
                nc.vector.tensor_reduce(
                    out_ap, rblk[:].rearrange("p (b d) -> p b d", d=D),
                    axis=mybir.AxisListType.X, op=Alu.add)

        def emit_reduce_pool(out_ap, rblk, rows, tag):
            # d-axis log2 add-tree on Pool via scalar_tensor_tensor
            # (TensorScalarPtr: 0.6 efficiency vs TensorTensor-Add's 0.42)
            v = rblk[:].rearrange("p (b d) -> p b d", d=D)
            for w in (8, 4, 2):
                tmp = redp.tile([rows, RB * NB * w], f16, tag=f"{tag}{w}",
                                name=f"{tag}{w}")
                tv = tmp[:].rearrange("p (b d) -> p b d", d=w)
                nc.gpsimd.tensor_add(tv, v[:, :, 0:w], v[:, :, w:2 * w])
                v = tv
            nc.gpsimd.tensor_add(out_ap.rearrange("p (b d) -> p b d", d=1),
                                 v[:, :, 0:1], v[:, :, 1:2])

        # Weight loads, ordered by first use.
        V0T = const.tile([F, 256 * NC0], f8)
        nc.sync.dma_start(out=V0T[:], in_=V0T_d.ap())

        # x hi/lo fp8 two-plane working buffers (4-deep rotation):
        # rows 0:39 x (hi plane cols 0:NT, lo cols NT:2NT), rows 39:64 zero
        # forever, rows 64:128 nh hi/lo written per tile.
        NTB = 4
        NXB = 5
        wz = const.tile([32, NT], f8, name="wz")
        nc.gpsimd.memset(wz[:], 0.0)
        t8 = [const.tile([T1, 2 * NT], f8, name=f"t8_{i}") for i in range(NTB)]
        x2o = [const.tile([XR, NT], f16, name=f"x2o_{i}") for i in range(NXB)]
        for i in (3, 0, 1, 2):
            nc.gpsimd.memset(t8[i][32:NH0, :], 0.0)
            nc.gpsimd.memset(t8[i][NH0:T1, NT:2 * NT], 0.0)
        for j in (0, 1, 2):
            nc.sync.dma_start(out=t8[j][0:F, 0:NT],
                              in_=xhi_d.ap()[:, j * NT:(j + 1) * NT])
            nc.sync.dma_start(out=t8[j][0:F, NT:2 * NT],
                              in_=xlo_d.ap()[:, j * NT:(j + 1) * NT])
            nc.sync.dma_start(out=x2o[j][:],
                              in_=x2o_d.ap()[:, j * NT:(j + 1) * NT])

        C0T = const.tile([128, 128 * NC0], f16)
        nc.sync.dma_start(out=C0T[:], in_=C0T_d.ap())
        G0T = const.tile([XR, 128], f16)
        nc.sync.dma_start(out=G0T[:], in_=G0T_d.ap())

        HCV = 256 * NC1 // 2
        HCC = 128 * NC1 // 2
        V1T = const.tile([T1, 256 * NC1], f8)
        C1T = const.tile([128, 128 * NC1], f16)
        nc.sync.dma_start(out=V1T[:, 0:HCV], in_=V1T_d.ap()[:, 0:HCV])
        nc.sync.dma_start(out=C1T[:, 0:HCC], in_=C1T_d.ap()[:, 0:HCC])
        nc.sync.dma_start(out=V1T[:, HCV:], in_=V1T_d.ap()[:, HCV:])
        nc.sync.dma_start(out=C1T[:, HCC:], in_=C1T_d.ap()[:, HCC:])
        G1T = const.tile([XR, 128], f16)
        nc.sync.dma_start(out=G1T[:], in_=G1T_d.ap())

        out0 = const.tile([H1, BL], f16)
        out1 = const.tile([L1, BL], f16)

        # Warmup matmuls on zeroed pad rows: keep PE busy through the input
        # DMA latency, pre-pay the p-state ramp.
        warm = yps.tile([32, NT], f32, tag="y1", name="warm")
        for _ in range(5):
            nc.tensor.matmul(warm[:], wz[0:32, 0:32], wz[:, 0:NT],
                             start=True, stop=True)

        sq0_ref = {}
        sq1_ref = {}
        y0_ref = {}
        y1_ref = {}
        r0b_ref = {}
        r1b_ref = {}

        def v_pass(ps2, Vt, tk, rows, i, kc, half):
            lhs = Vt[:, i * 256:(i + 1) * 256] \
                .rearrange("p (two m) -> p two m", two=2)[:, :, 0:kc]
            rhs = tk[0:rows, :].rearrange("p (two n) -> p two n", two=2)
            nc.tensor.matmul(ps2[0:kc, half * NT:(half + 1) * NT],
                             lhs, rhs, start=True, stop=True, perf_mode=DR)

        def emit_v0(k):
            tk = t8[k % NTB]
            sq0 = sqp.tile([128, NC0 * NT], f16, tag="sq0", name="sq0")
            sq0_ref[k] = sq0
            pend = []
            for p in range(NP0):
                ps2 = sps.tile([128, 2 * NT], f32, tag="ps2", name="ps2")
                for half in (0, 1):
                    i = 2 * p + half
                    o, kc = CH0[i]
                    v_pass(ps2, V0T, tk, F, i, kc, half)
                pend.append((sq0, slice(2 * p * NT, (2 * p + 2) * NT), ps2, p))
            return pend

        def emit_c0g0(j):
            # C0/G0 accumulation for tile j (squares from last iteration),
            # then the nh hi/lo chain into t8[j] and the r0 block relu.
            y0 = yps.tile([L0, NT], f32, tag="y0", name="y0")
            y0_ref[j] = y0
            for n, i in enumerate(_cons_order(NC0, 0)):
                o, kc = CH0[i]
                nc.tensor.matmul(y0[:], C0T[0:kc, i * 128:(i + 1) * 128],
                                 sq0_ref[j][0:kc, i * NT:(i + 1) * NT],
                                 start=(n == 0), stop=False)
            nc.tensor.matmul(y0[:], G0T[:], x2o[j % NXB][:],
                             start=False, stop=True)
            sq0_ref.pop(j)
            # vector ops emitted immediately so they lead the engine queues:
            # nh_hi = fp8(relu(y0)); nh_lo = (y0 max 0) - nh_hi (DVE; PSUM
            # operand is exempt from the same-start-partition rule).
            tk = t8[j % NTB]
            nc.scalar.activation(tk[NH0:T1, 0:NT], y0[0:H1, :], Relu)
            if j % RB == 0:
                r0b_ref[j // RB] = rbp.tile([H1, RB * NT], f16, tag="r0b",
                                            name="r0b")
            nc.vector.tensor_scalar_max(
                r0b_ref[j // RB][:, (j % RB) * NT:(j % RB + 1) * NT],
                y0[H1:L0, :], 0.0)

        def emit_v1(j, interleave):
            # V1 pairs for tile j, interleaved with the supplied consumer
            # callables (C1/G1 passes of tile j-1).
            tk = t8[j % NTB]
            sq1 = sqp.tile([128, NC1 * NT], f16, tag="sq1", name="sq1")
            sq1_ref[j] = sq1
            ci = 0
            for p in range(NP1):
                ps2 = sps.tile([128, 2 * NT], f32, tag="ps2", name="ps2")
                for half in (0, 1):
                    i = 2 * p + half
                    o, kc = CH1[i]
                    v_pass(ps2, V1T, tk, T1, i, kc, half)
                emit_square_pair(sq1, slice(2 * p * NT, (2 * p + 2) * NT),
                                 ps2, NP0 + p)
                take = ((p + 1) * len(interleave)) // NP1 - ci
                for _ in range(take):
                    interleave[ci]()
                    ci += 1
            for f in interleave[ci:]:
                f()

        def c1_ops(j):
            # consumer callables: C1 passes + G1 + r1 relu for tile j
            y1 = yps.tile([L1, NT], f32, tag="y1", name="y1")
            y1_ref[j] = y1
            ops = []
            for n, i in enumerate(_cons_order(NC1, NP0)):
                o, kc = CH1[i]
                def cpass(i=i, kc=kc, n=n):
                    nc.tensor.matmul(y1[:], C1T[0:kc, i * 128:(i + 1) * 128],
                                     sq1_ref[j][0:kc, i * NT:(i + 1) * NT],
                                     start=(n == 0), stop=False)
                ops.append(cpass)

            def gpass():
                nc.tensor.matmul(y1[:], G1T[:], x2o[j % NXB][:],
                                 start=False, stop=True)
                sq1_ref.pop(j)
                if j % RB == 0:
                    r1b_ref[j // RB] = rbp.tile([L1, RB * NT], f16, tag="r1b",
                                                name="r1b")
                nc.vector.tensor_scalar_max(
                    r1b_ref[j // RB][:, (j % RB) * NT:(j % RB + 1) * NT],
                    y1[:], 0.0)
            ops.append(gpass)
            return ops

        def c1_ops_halves(j):
            # final tile: run C1/G1 in column halves so the relu/reduce/DMA
            # drain runs on half-width and overlaps the second half's PE.
            # y0's bank is free in the drain iteration, so the second half
            # borrows its tag.
            HW = NT // 2
            y1h = [yps.tile([L1, HW], f32, tag="y1", name="y1a"),
                   yps.tile([L1, HW], f32, tag="y0", name="y1b")]
            r1t = rbp.tile([L1, NT], f16, tag="r1b", name="r1h")
            ops = []
            order = _cons_order(NC1, NP0)
            for h in (0, 1):
                hs = slice(h * HW, (h + 1) * HW)
                for n, i in enumerate(order):
                    def cpass(i=i, n=n, h=h, hs=hs):
                        o, kc = CH1[i]
                        nc.tensor.matmul(
                            y1h[h][:], C1T[0:kc, i * 128:(i + 1) * 128],
                            sq1_ref[j][0:kc, i * NT + hs.start:
                                       i * NT + hs.stop],
                            start=(n == 0), stop=False)
                    ops.append(cpass)

                def gpass(h=h, hs=hs):
                    nc.tensor.matmul(y1h[h][:], G1T[:],
                                     x2o[j % NXB][:, hs],
                                     start=False, stop=True)
                    if h == 1:
                        sq1_ref.pop(j)
                    nc.vector.tensor_scalar_max(r1t[:, hs], y1h[h][:], 0.0)
                    csl = slice(j * NB + h * (NB // 2),
                                j * NB + (h + 1) * (NB // 2))
                    emit_reduce(out1[:, csl], r1t[:, hs])
                    nc.sync.dma_start(out=out_d.ap()[H1:, csl],
                                      in_=out1[:, csl])
                ops.append(gpass)
            return ops

        for k in range(NTILES + 2):
            v0t = k if k < NTILES else None            # V0 tile
            c0t = k - 1 if 0 <= k - 1 < NTILES else None
            c1t = k - 2 if 0 <= k - 2 < NTILES else None
            v1t = k - 1 if 0 <= k - 1 < NTILES else None

            # C0/G0 first: starts the nh chain early and delays V0's ring
            # allocations so the previous iteration's last squares drain.
            if c0t is not None:
                emit_c0g0(c0t)
            pend_sq0 = emit_v0(v0t) if v0t is not None else []
            for args in pend_sq0:
                emit_square_pair(*args)

            # prologue fillers: k=0 has no C work, k=1 has no C1 work and
            # V1(0) waits on the nh(0) vector chain.  PE runs in emission
            # order, so fillers go here to bridge those bubbles.
            if k in (0, 1):
                warm2 = yps.tile([32, NT], f32, tag="y1",
                                 name=f"warm2_{k}")
                for _ in range(7 if k == 0 else 5):
                    nc.tensor.matmul(warm2[:], wz[0:32, 0:32], wz[:, 0:NT],
                                     start=True, stop=True)

            cons = c1_ops(c1t) if c1t is not None else []
            if v1t is not None:
                # C1 prefix gives the nh chain time before the first V1 pass
                npre = min(6, len(cons))
                for f in cons[:npre]:
                    f()
                emit_v1(v1t, cons[npre:])
            else:
                for f in cons:
                    f()

            # block reduces + output DMA: r0 blocks complete at k = 4,8,12,16
            # (r0(j) written at iteration j+1), r1 blocks at k = 5,9,13,17.
            if c0t is not None and c0t % RB == RB - 1:
                blk = c0t // RB
                osl = slice(blk * RB * NB, (blk + 1) * RB * NB)
                emit_reduce(out0[:, osl], r0b_ref.pop(blk))
                nc.sync.dma_start(out=out_d.ap()[0:H1, osl], in_=out0[:, osl])
            if c1t is not None and c1t % RB == RB - 1:
                blk = c1t // RB
                osl = slice(blk * RB * NB, (blk + 1) * RB * NB)
                emit_reduce(out1[:, osl], r1b_ref.pop(blk))
                nc.sync.dma_start(out=out_d.ap()[H1:, osl], in_=out1[:, osl])

            # prefetch x hi/lo + x^2 (t8: 4-slot ring, slot (v0t+3)%4 was
            # last read by V1(v0t-1) this iteration; x2o: 5-slot ring)
            if v0t is not None and v0t + 3 < NTILES:
                nxt = v0t + 3
                nc.sync.dma_start(out=t8[nxt % NTB][0:F, 0:NT],
                                  in_=xhi_d.ap()[:, nxt * NT:(nxt + 1) * NT])
                nc.sync.dma_start(out=t8[nxt % NTB][0:F, NT:2 * NT],
                                  in_=xlo_d.ap()[:, nxt * NT:(nxt + 1) * NT])
                nc.sync.dma_start(out=x2o[nxt % NXB][:],
                                  in_=x2o_d.ap()[:, nxt * NT:(nxt + 1) * NT])

    nc.compile()
    _NC_CACHE[key] = nc
    return nc


def _run(inputs, trace=False):
    from concourse.bass_utils import run_bass_kernel_spmd

    x = np.asarray(inputs["x"], np.float32)
    w = _host_weights(inputs["W0"], inputs["b0"], inputs["W1"], inputs["b1"])
    nc = _build_nc()

    in_maps = []
    for c in range(NCORES):
        xs = x[c * BL:(c + 1) * BL]                          # [512, 39, 16]
        cols = np.ascontiguousarray(
            xs.transpose(1, 0, 2).reshape(F, NCOL)).astype(np.float32)
        xhi = cols.astype(F8NP)
        xlo = (cols - xhi.astype(np.float32)).astype(F8NP)
        x2o = np.empty((XR, NCOL), np.float16)
        x2o[:F] = (cols * cols).astype(np.float16)
        x2o[F] = 1.0
        m = {"xhi": xhi, "xlo": xlo, "x2o": x2o}
        m.update(w)
        in_maps.append(m)

    res = run_bass_kernel_spmd(nc, in_maps, core_ids=list(range(NCORES)),
                               trace=trace)
    out = np.empty((B, L0 - H1 + L1), np.float32)
    for c in range(NCORES):
        out[c * BL:(c + 1) * BL] = res.results[c]["out"].T.astype(np.float32)
    return out, res


def kernel(**inputs):
    out, _ = _run(inputs)
    return out
